# revision 6
# baseline (speedup 1.0000x reference)
"""CRF NLL (mean) loss kernel for Trainium2, 8 NeuronCores.

Strategy (hardcoded for B=256, S=512, T=64):
  - Data-parallel over batch: 32 sequences per core.
  - Denominator (log-partition) on device via a BIDIRECTIONAL exp-space scan:
    forward chain from s=0 and backward chain from s=511 run fused as one
    [128, 32] state (top 64 partitions = fwd alpha^T, bottom = bwd beta^T),
    meeting in the middle after 255 rows:
        rhs_{j+1} = (WD.T @ rhs_j) * E_j
    with WD = blockdiag(expM, expM.T) bf16 stationary, E_j the stacked
    transposed emission exponentials exp(em - CBAR) in bf16.  The CBAR
    prescale keeps values in f32/bf16 range with NO renormalization
    (validated: max denom err 0.04 nats vs f64 at |denom|~2400).
  - Emissions are packed on host as fwd half [s=0..255] and REVERSED bwd
    half [s=511..256] so both chains read ascending; ACT exps them into
    paired 128-wide blocks; the DMA xbar transposes each [32,128] block
    to a [128,32] E tile (dtype bf16 as the xbar requires).
  - Z_b = sum_t alpha_255[t,b] * beta_255[t,b] computed on host in f64 from
    the two [128,32] outputs (rhs_255 bf16, ps_255 f32).
  - Numerator (gold path score) on host in numpy (gathers; ~0.3% of FLOPs).
  - Final mean on host.
"""

import sys

import numpy as np
import ml_dtypes

sys.path.insert(0, "/opt/trn_rl_repo")

B, S, T = 256, 512, 64
NCORES = 8
BL = B // NCORES   # 32 sequences per core
HALF = S // 2      # 256 steps per chain direction
ROWS = HALF - 1    # 255 chain rows with an emission mul
CBAR = 4.7         # exp prescale: exp(em - CBAR); log Z += S*CBAR on host

_CACHE = {}


def _build_nc():
    import concourse.bass as bass
    import concourse.mybir as mybir
    from concourse import tile

    AF = mybir.ActivationFunctionType
    f32 = mybir.dt.float32
    bf16 = mybir.dt.bfloat16

    nc = bass.Bass()
    emF_d = nc.dram_tensor("emF", [BL, HALF * T], f32, kind="ExternalInput")
    emB_d = nc.dram_tensor("emB", [BL, HALF * T], f32, kind="ExternalInput")
    wd_d = nc.dram_tensor("wd", [2 * T, 2 * T], bf16, kind="ExternalInput")
    scol_d = nc.dram_tensor("scol", [2 * T, 1], f32, kind="ExternalInput")
    orhs_d = nc.dram_tensor("orhs", [2 * T, BL], bf16, kind="ExternalOutput")
    ops_d = nc.dram_tensor("ops", [2 * T, BL], f32, kind="ExternalOutput")

    # pair tile c holds row-blocks: c=0 -> rows 0..62, c>=1 -> rows
    # 64c-1..64c+62; block [j][0:64] = exp(emF step j+1), [64:128] = exp(emB
    # step j+1).  A separate init tile holds [exp(emF 0) | exp(emB 0)].
    with tile.TileContext(nc) as tc:
        with (
            tc.tile_pool(name="consts", bufs=1) as consts,
            tc.tile_pool(name="emc", bufs=3) as emp,
            tc.tile_pool(name="pair", bufs=1) as pairp,
            tc.tile_pool(name="et", bufs=1) as etp,
            tc.tile_pool(name="rhs", bufs=4) as rp,
            tc.tile_pool(name="fin", bufs=1) as finp,
            tc.tile_pool(name="psum", bufs=4, space="PSUM") as psp,
        ):
            wd = consts.tile([2 * T, 2 * T], bf16)
            scol = consts.tile([2 * T, 1], f32)
            nbias = consts.tile([BL, 1], f32)
            nc.sync.dma_start(wd[:], wd_d[:])
            nc.sync.dma_start(scol[:], scol_d[:])
            nc.vector.memset(nbias[:], -CBAR)

            pair0 = pairp.tile([BL, 63, 2 * T], bf16, tag="pair0")
            pairs = [pair0] + [
                pairp.tile([BL, 64, 2 * T], bf16, tag=f"pair{c}",
                           name=f"pair{c}")
                for c in (1, 2, 3)
            ]
            pinit = pairp.tile([BL, 2 * T], bf16, tag="pinit")

            # Emission staging.  em-chunk DMAs issue from the ACT hwdge front
            # (their slot-recycle waits are on ACT's own exps, so no
            # cross-engine stall); xbar transposes issue from SP.  Emission
            # order = consumption order so the in-order queues never head-of-
            # line block the chain: init path first, then pair tiles 0..3.
            chF0 = emp.tile([BL, 64 * T], f32, tag="emc")
            nc.scalar.dma_start(chF0[:], emF_d[:, 0:64 * T])
            chB0 = emp.tile([BL, 64 * T], f32, tag="emc")
            nc.scalar.dma_start(chB0[:], emB_d[:, 0:64 * T])
            vF0 = chF0[:].rearrange("p (s t) -> p s t", t=T)
            vB0 = chB0[:].rearrange("p (s t) -> p s t", t=T)
            nc.scalar.activation(pinit[:, 0:T], vF0[:, 0, :], AF.Exp,
                                 bias=nbias[:])
            nc.scalar.activation(pinit[:, T:2 * T], vB0[:, 0, :], AF.Exp,
                                 bias=nbias[:])
            einit = etp.tile([2 * T, BL], bf16, tag="einit")
            nc.sync.dma_start(einit[:], pinit[:], transpose=True)
            rhs = rp.tile([2 * T, BL], bf16, tag="rhs")
            nc.vector.tensor_scalar_mul(rhs[:], einit[:], scol[:])

            # pair tile exps + one blocked xbar transpose per tile:
            # [32, nblk*128] -> [128, nblk, 32] = all E tiles of that chunk.
            nc.scalar.activation(pairs[0][:, :, 0:T], vF0[:, 1:64, :],
                                 AF.Exp, bias=nbias[:])
            nc.scalar.activation(pairs[0][:, :, T:2 * T], vB0[:, 1:64, :],
                                 AF.Exp, bias=nbias[:])
            ets = []
            et0 = etp.tile([2 * T, 63, BL], bf16, tag="et0")
            nc.sync.dma_start(et0[:], pairs[0][:], transpose=True)
            ets.append(et0)
            for c in (1, 2, 3):
                for src, lo in ((emF_d, 0), (emB_d, T)):
                    ch = emp.tile([BL, 64 * T], f32, tag="emc")
                    nc.scalar.dma_start(ch[:], src[:, c * 64 * T:(c + 1) * 64 * T])
                    nc.scalar.activation(
                        pairs[c][:, :, lo:lo + T],
                        ch[:].rearrange("p (s t) -> p s t", t=T)[:],
                        AF.Exp, bias=nbias[:])
                ett = etp.tile([2 * T, 64, BL], bf16, tag=f"et{c}",
                               name=f"et{c}")
                nc.sync.dma_start(ett[:], pairs[c][:], transpose=True)
                ets.append(ett)

            for j in range(ROWS):
                c = 0 if j <= 62 else (j + 1) // 64
                blk = j if c == 0 else j - (64 * c - 1)
                ps = psp.tile([2 * T, BL], f32, tag="ps")
                nc.tensor.matmul(ps[:], wd[:], rhs[:])
                rhs2 = rp.tile([2 * T, BL], bf16, tag="rhs")
                nc.vector.tensor_mul(rhs2[:], ps[:], ets[c][:, blk, :])
                rhs = rhs2

            # final matmul row (no emission mul); outputs to host
            ps = psp.tile([2 * T, BL], f32, tag="ps")
            nc.tensor.matmul(ps[:], wd[:], rhs[:])
            fin = finp.tile([2 * T, BL], f32)
            nc.scalar.copy(fin[:], ps[:])
            nc.sync.dma_start(orhs_d[:], rhs[:])
            nc.sync.dma_start(ops_d[:], fin[:])

    _split_multi_waits(nc)
    return nc


def _split_multi_waits(nc):
    # This toolchain's walrus rejects >1 sync-wait command per instruction
    # ("Too many sync wait commands").  Hoist all but the last wait of any
    # multi-wait instruction onto same-engine NoOps inserted just before it.
    import concourse.mybir as mybir

    for f in nc.m.functions:
        for bb in f.blocks:
            il = bb.instructions
            i = 0
            while i < len(il):
                inst = il[i]
                si = getattr(inst, "sync_info", None)
                if si is not None and len(si.on_wait) > 1:
                    waits = list(si.on_wait)
                    for k, w in enumerate(waits[:-1]):
                        nop = mybir.InstNoOp(
                            name=f"{inst.name}-w{k}", ins=[], outs=[])
                        nop.engine = inst.engine
                        nop.sync_info = mybir.SyncInfo(
                            on_wait=[w], on_update=[])
                        il.insert(i, nop)
                        i += 1
                    inst.sync_info = mybir.SyncInfo(
                        on_wait=[waits[-1]], on_update=list(si.on_update))
                i += 1


def _numerator(emissions, tags, mask, start_transitions, end_transitions, transitions):
    # Gold-path score per sequence, f64 accumulation on host.
    tg = tags.astype(np.int64)
    em = emissions.astype(np.float64)
    maskf = mask.astype(np.float64)
    b_idx = np.arange(B)
    emit = np.take_along_axis(em, tg[:, :, None], axis=2)[..., 0]      # [B, S]
    trans_sc = transitions.astype(np.float64)[tg[:, :-1], tg[:, 1:]]   # [B, S-1]
    score = start_transitions.astype(np.float64)[tg[:, 0]] + emit[:, 0]
    score = score + np.sum((trans_sc + emit[:, 1:]) * maskf[:, 1:], axis=1)
    seq_ends = np.sum(mask != 0, axis=1).astype(np.int64) - 1
    last_tags = tg[b_idx, seq_ends]
    score = score + end_transitions.astype(np.float64)[last_tags]
    return score  # [B] f64


def _denominator_host(emissions, mask, start_transitions, end_transitions, transitions):
    # General-mask fallback (never hit for the spec'd all-ones mask): scaled
    # exp-space forward scan in f64 on host.
    em = emissions.astype(np.float64)
    Mx = np.exp(transitions.astype(np.float64))
    alpha = np.exp(start_transitions.astype(np.float64)[None, :] + em[:, 0, :])
    logz = np.zeros(B)
    for s in range(1, S):
        nxt = (alpha @ Mx) * np.exp(em[:, s, :])
        m = mask[:, s].astype(bool)
        alpha = np.where(m[:, None], nxt, alpha)
        c = alpha.sum(axis=1)
        alpha /= c[:, None]
        logz += np.log(c)
    final = alpha * np.exp(end_transitions.astype(np.float64))[None, :]
    return logz + np.log(final.sum(axis=1))


def _run_device(emissions, start_transitions, end_transitions, transitions,
                trace=False):
    from concourse.bass_utils import run_bass_kernel_spmd

    if "nc" not in _CACHE:
        _CACHE["nc"] = _build_nc()
    nc = _CACHE["nc"]

    expM = np.exp(transitions.astype(np.float64))
    wd = np.zeros((2 * T, 2 * T), dtype=np.float64)
    wd[0:T, 0:T] = expM
    wd[T:2 * T, T:2 * T] = expM.T
    wd = wd.astype(ml_dtypes.bfloat16)
    scol = np.concatenate([
        np.exp(start_transitions.astype(np.float64)),
        np.exp(end_transitions.astype(np.float64)),
    ]).reshape(2 * T, 1).astype(np.float32)

    em = np.asarray(emissions, dtype=np.float32)
    in_maps = []
    for c in range(NCORES):
        sh = em[c * BL:(c + 1) * BL]                       # [BL, S, T]
        emF = np.ascontiguousarray(sh[:, :HALF]).reshape(BL, HALF * T)
        emB = np.ascontiguousarray(sh[:, :HALF - 1:-1]).reshape(BL, HALF * T)
        in_maps.append({"emF": emF, "emB": emB, "wd": wd, "scol": scol})
    res = run_bass_kernel_spmd(nc, in_maps, list(range(NCORES)), trace=trace)

    denoms = []
    for c in range(NCORES):
        top = res.results[c]["orhs"][0:T, :].astype(np.float64)   # alpha_255
        bot = res.results[c]["ops"][T:2 * T, :].astype(np.float64)  # beta_255
        Z = (top * bot).sum(axis=0)                               # [BL]
        denoms.append(np.log(Z) + S * CBAR)
    return np.concatenate(denoms), res


def kernel(emissions, tags, mask, start_transitions, end_transitions, transitions):
    emissions = np.asarray(emissions, dtype=np.float32)
    tags = np.asarray(tags)
    mask = np.asarray(mask)
    start_transitions = np.asarray(start_transitions, dtype=np.float32)
    end_transitions = np.asarray(end_transitions, dtype=np.float32)
    transitions = np.asarray(transitions, dtype=np.float32)

    score = _numerator(emissions, tags, mask, start_transitions,
                       end_transitions, transitions)

    if np.all(mask != 0):
        denom, _ = _run_device(emissions, start_transitions, end_transitions,
                               transitions)
    else:
        denom = _denominator_host(emissions, mask, start_transitions,
                                  end_transitions, transitions)

    llh = denom.astype(np.float64) - score
    return np.float32(np.mean(llh))
